# revision 10
# baseline (speedup 1.0000x reference)
"""Trainium2 Bass kernel for GQA (nn_GQA_28561532518475).

8 cores = 4 batches x 2 kv-head halves.  perm is folded into the weights on
the host (Wq cols -> slot order * scale, Wk/Wv rows by argsort(perm), Wp rows
by perm), so the device kernel is a plain GQA.

v2 changes vs baseline:
- Scores matmuls for the two grouped q-heads (g=0 rows 0:64, g=1 rows 64:128)
  are emitted adjacently so they run CONCURRENTLY via PE row tiling
  (tile_position auto-derived from base_partition) -> ~2x on the scores GEMM.
- Both heads of a kv group stream through the same i-loop, with exp of the
  [128,1024] score tiles split between the Scalar engine (hardware exp) and
  the Vector engine (Schraudolph exp2 bit-trick: one tensor_scalar affine with
  int32 output, bitcast back to fp32) to break the ACT 1-elem/lane/cycle
  bottleneck (~220us if all exp on ACT).
- v projection moving operand padded to 256 cols (fp32r below 256 runs at
  1/4 rate on a warm PE).
- V for all 3 kv heads lives in one [128, PT, NKV, HD+1] tile with a ones
  column so the attention matmul also emits the softmax denominator.
"""

import numpy as np

B, P, C = 4, 2048, 768
H, HK, HD, GS = 12, 6, 64, 2
SCALE = HD ** -0.5
NKV = 3          # kv heads per core
NH = 6           # q heads per core
KT = C // 128    # 6 contraction tiles
PT = P // 128    # 16 row tiles
QB = 1024        # q-block width for attention
NQB = P // QB    # 2

# Schraudolph exp2 constants (bf16 variant): exp(s) ~= bitcast_bf16(
# round_i16(s*log2e*2^7 + ((127 - 0.0564) * 2^7))).  c = -0.0564 zeroes the
# mean relative error of the mantissa-linear-interp over a uniform
# fractional part.
_EXP_S1 = float(1.4426950408889634 * (1 << 7))
_EXP_S2 = float((127 << 7) - 0.0564 * (1 << 7))

_cached_nc = None


def _build_program():
    global _cached_nc
    if _cached_nc is not None:
        return _cached_nc

    import concourse.bass as bass
    import concourse.mybir as mybir
    import concourse.tile as tile
    from concourse import bacc

    fp32 = mybir.dt.float32
    fp32r = mybir.dt.float32r
    bf16 = mybir.dt.bfloat16
    i16 = mybir.dt.int16
    EXP = mybir.ActivationFunctionType.Exp
    MULT = mybir.AluOpType.mult
    ADD = mybir.AluOpType.add

    nc = bacc.Bacc("TRN2", target_bir_lowering=False, debug=False)
    xT = nc.dram_tensor("xT", [C, P], fp32r, kind="ExternalInput").ap()
    wq = nc.dram_tensor("wq", [C, 384], fp32r, kind="ExternalInput").ap()
    wkd = nc.dram_tensor("wkd", [C, 384], fp32r, kind="ExternalInput").ap()
    wv = nc.dram_tensor("wv", [C, 256], fp32r, kind="ExternalInput").ap()
    wp = nc.dram_tensor("wp", [384, C], fp32r, kind="ExternalInput").ap()
    y = nc.dram_tensor("y", [P, C], fp32, kind="ExternalOutput").ap()
    rcd = nc.dram_tensor("rcd", [NH * NQB, QB], fp32).ap()
    rcd2 = nc.dram_tensor("rcd2", [NH * NQB, QB], fp32).ap()

    with tile.TileContext(nc) as tc:
        from contextlib import ExitStack

        with ExitStack() as ctx:
            wpool = ctx.enter_context(tc.tile_pool(name="weights", bufs=1))
            qkvp = ctx.enter_context(tc.tile_pool(name="qkv", bufs=1))
            outp = ctx.enter_context(tc.tile_pool(name="outT", bufs=1))
            epool = ctx.enter_context(tc.tile_pool(name="E", bufs=4))
            nrmp = ctx.enter_context(tc.tile_pool(name="norm", bufs=1))
            ysbp = ctx.enter_context(tc.tile_pool(name="ysb", bufs=3))

            # x kept as 4 column-quarter tiles so consumers start after the
            # first quarter's DMA instead of the full 6.3MB transfer
            xts = [wpool.tile([128, KT, 512], fp32r, name=f"xt{q}") for q in range(4)]
            for q in range(4):
                for kc in range(KT):
                    nc.sync.dma_start(
                        xts[q][:, kc, :],
                        xT[kc * 128 : (kc + 1) * 128, q * 512 : (q + 1) * 512],
                    )
            wv_sb = wpool.tile([128, KT, 256], fp32r)
            nc.sync.dma_start(wv_sb[:], wv.rearrange("(t p) n -> p t n", p=128))
            wq_sb = wpool.tile([128, KT, 384], fp32r)
            nc.sync.dma_start(wq_sb[:], wq.rearrange("(t p) n -> p t n", p=128))
            wkd_sb = wpool.tile([128, KT, 384], fp32r)
            nc.sync.dma_start(wkd_sb[:], wkd.rearrange("(t p) n -> p t n", p=128))
            wp_sb = wpool.tile([128, 3, C], fp32r)
            nc.sync.dma_start(wp_sb[:], wp.rearrange("(t p) n -> p t n", p=128))

            qts = [qkvp.tile([128, P], fp32r, name=f"qt{m}", tag=f"qt{m}") for m in range(NKV)]
            kts = [qkvp.tile([128, P], fp32r, name=f"kt{m}", tag=f"kt{m}") for m in range(NKV)]
            # all 3 kv heads' V in one tile: [128, PT, NKV, HD+1], col HD = ones
            vex = qkvp.tile([128, PT, NKV, HD + 1], bf16, name="vex", tag="vex")
            nc.vector.memset(vex[:, :, :, HD], 1.0)
            outTs = [outp.tile([128, P], fp32r, name=f"oT{m}", tag=f"oT{m}") for m in range(NKV)]

            expctr = [0]

            with tc.tile_pool(name="mm_ps", bufs=2, space="PSUM") as sps, tc.tile_pool(
                name="o_ps", bufs=1, space="PSUM"
            ) as ops:

                def qk_proj(kv):
                    for di, (w_sb, dest) in enumerate(((wq_sb, qts[kv]), (wkd_sb, kts[kv]))):
                        for nb in range(4):
                            ps = sps.tile([128, 512], fp32, name="pj", tag="s")
                            for kc in range(KT):
                                nc.tensor.matmul(
                                    ps[:],
                                    w_sb[:, kc, kv * 128 : (kv + 1) * 128],
                                    xts[nb][:, kc, :],
                                    start=(kc == 0),
                                    stop=(kc == KT - 1),
                                )
                            # alternate the psum->sbuf copies between DVE and
                            # ACT to balance engine load
                            if (di * 4 + nb) % 2 == 0:
                                nc.vector.tensor_copy(
                                    dest[:, nb * 512 : (nb + 1) * 512], ps[:]
                                )
                            else:
                                nc.scalar.copy(
                                    dest[:, nb * 512 : (nb + 1) * 512], ps[:]
                                )

                def v_proj_all():
                    for i in range(PT):
                        ps = sps.tile([128, 256], fp32, name="vp", tag="s")
                        for kc in range(KT):
                            nc.tensor.matmul(
                                ps[:],
                                xts[i // 4][:, kc, (i % 4) * 128 : (i % 4 + 1) * 128],
                                wv_sb[:, kc, :],
                                start=(kc == 0),
                                stop=(kc == KT - 1),
                            )
                        psv = ps.rearrange("p (h d) -> p h d", h=4)
                        # one strided copy: [128, 3, 64] -> vex[:, i, :, 0:64]
                        nc.scalar.copy(vex[:, i, :, 0:HD], psv[:, 0:NKV, :])

                def emit_exp(e_i16, s_ps, on_dve):
                    if on_dve:
                        nc.vector.tensor_scalar(
                            e_i16[:], s_ps[:], _EXP_S1, _EXP_S2, MULT, ADD
                        )
                    else:
                        nc.scalar.activation(e_i16.bitcast(bf16), s_ps[:], EXP)

                def emit_av(kv, obs, i, es):
                    for nb in range(QB // 512):
                        for g in range(GS):
                            nc.tensor.matmul(
                                obs[g][:, nb * 512 : (nb + 1) * 512],
                                vex[:, i, kv, :],
                                es[g].bitcast(bf16)[:, nb * 512 : (nb + 1) * 512],
                                start=(i == 0),
                                stop=(i == PT - 1),
                            )

                def attention(kv):
                    for jq in range(NQB):
                        q0 = jq * QB
                        obs = [
                            ops.tile([HD + 1, QB], fp32, name=f"ob{g}", tag=f"ob{g}")
                            for g in range(GS)
                        ]
                        prev = None
                        for i in range(PT):
                            sts = [
                                sps.tile([128, QB], fp32, name=f"sc{g}", tag="s")
                                for g in range(GS)
                            ]
                            # row-tiled concurrent score matmuls: g=0 on PE
                            # rows 0:64, g=1 on rows 64:128, adjacent in the
                            # instruction stream
                            for nb in range(QB // 512):
                                for g in range(GS):
                                    gp = slice(g * 64, (g + 1) * 64)
                                    nc.tensor.matmul(
                                        sts[g][:, nb * 512 : (nb + 1) * 512],
                                        kts[kv][gp, i * 128 : (i + 1) * 128],
                                        qts[kv][gp, q0 + nb * 512 : q0 + (nb + 1) * 512],
                                        start=True,
                                        stop=True,
                                    )
                            es = []
                            for g in range(GS):
                                e = epool.tile([128, QB], i16, tag="e")
                                # one exp on ACT, one on DVE each i; swap
                                # parity so each head is 50% Schraudolph
                                emit_exp(e, sts[g], on_dve=((g + i) % 2 == 1))
                                es.append(e)
                            # AV for the previous i is emitted after this i's
                            # scores so the PE FIFO has work while exp(i) runs
                            if prev is not None:
                                emit_av(kv, obs, prev[0], prev[1])
                            prev = (i, es)
                        emit_av(kv, obs, prev[0], prev[1])
                        # normalize: free the ob psum quickly via an ACT copy
                        # to SBUF, then softmax sums -> DRAM -> lane-spread
                        # reciprocal -> DRAM -> partition-broadcast read, and
                        # a GpSimd multiply (all-SBUF operands)
                        for g in range(GS):
                            ob = obs[g]
                            h = 2 * kv + g
                            u = h * NQB + jq
                            osb = nrmp.tile([HD + 1, QB], fp32, tag=f"osb{g}")
                            if g == 0:
                                nc.scalar.copy(osb[:], ob[:])
                            else:
                                nc.vector.tensor_copy(osb[:], ob[:])
                            nc.sync.dma_start(rcd[u : u + 1, :], osb[HD : HD + 1, :])
                            rr = nrmp.tile([128, QB // 128], fp32, tag="rr")
                            lanes = bass.AP(
                                tensor=rcd.tensor,
                                offset=u * QB,
                                ap=[[QB // 128, 128], [1, QB // 128]],
                            )
                            nc.sync.dma_start(rr[:], lanes)
                            rr2 = nrmp.tile([128, QB // 128], fp32, tag="rr2")
                            nc.vector.reciprocal(rr2[:], rr[:])
                            lanes2 = bass.AP(
                                tensor=rcd2.tensor,
                                offset=u * QB,
                                ap=[[QB // 128, 128], [1, QB // 128]],
                            )
                            nc.sync.dma_start(lanes2, rr2[:])
                            rb = nrmp.tile([HD, QB], fp32, tag="rb")
                            bcast = bass.AP(
                                tensor=rcd2.tensor, offset=u * QB, ap=[[0, HD], [1, QB]]
                            )
                            nc.gpsimd.dma_start(rb[:], bcast)
                            if g == 0:
                                nc.gpsimd.tensor_mul(
                                    outTs[kv][0:HD, q0 : q0 + QB], osb[0:HD, :], rb[:]
                                )
                            else:
                                sc2 = nrmp.tile([HD, QB], fp32r, tag="sc2")
                                nc.gpsimd.tensor_mul(sc2[:], osb[0:HD, :], rb[:])
                                nc.sync.dma_start(
                                    outTs[kv][HD:128, q0 : q0 + QB], sc2[:]
                                )

                v_proj_all()
                qk_proj(0)
                attention(0)
                qk_proj(1)
                attention(1)
                qk_proj(2)
                attention(2)

            # ---------------- output projection ----------------
            with tc.tile_pool(name="y_ps", bufs=4, space="PSUM") as yps:
                for mt in range(PT):
                    for nh in range(2):
                        ps = yps.tile([128, 384], fp32, tag="y")
                        for kf in range(3):
                            nc.tensor.matmul(
                                ps[:],
                                outTs[kf][:, mt * 128 : (mt + 1) * 128],
                                wp_sb[:, kf, nh * 384 : (nh + 1) * 384],
                                start=(kf == 0),
                                stop=(kf == 2),
                            )
                        ysb = ysbp.tile([128, 384], fp32, tag="y")
                        nc.vector.tensor_copy(ysb[:], ps[:])
                        nc.sync.dma_start(
                            y[mt * 128 : (mt + 1) * 128, nh * 384 : (nh + 1) * 384],
                            ysb[:],
                        )

    nc.compile()
    _cached_nc = nc
    return nc


def _make_in_maps(x, Wq, Wk, Wv, Wp, perm):
    inv = np.argsort(perm)
    Wq_f = np.ascontiguousarray(
        Wq.reshape(C, H, HD)[:, perm, :].reshape(C, C) * SCALE
    )
    Wk_f = np.ascontiguousarray(Wk.reshape(H, HD, HK * HD)[inv].reshape(C, HK * HD))
    Wv_f = np.ascontiguousarray(Wv.reshape(H, HD, HK * HD)[inv].reshape(C, HK * HD))
    Wp_f = np.ascontiguousarray(Wp.reshape(H, HD, C)[perm].reshape(C, C))

    in_maps = []
    for core in range(8):
        b, half = core // 2, core % 2
        wk_half = Wk_f[:, half * 192 : (half + 1) * 192].reshape(C, NKV, 1, HD)
        wkd = np.ascontiguousarray(
            np.broadcast_to(wk_half, (C, NKV, 2, HD)).reshape(C, 384)
        )
        wv_pad = np.zeros((C, 256), np.float32)
        wv_pad[:, :192] = Wv_f[:, half * 192 : (half + 1) * 192]
        in_maps.append(
            {
                "xT": np.ascontiguousarray(x[b].T),
                "wq": np.ascontiguousarray(Wq_f[:, half * 384 : (half + 1) * 384]),
                "wkd": wkd,
                "wv": wv_pad,
                "wp": np.ascontiguousarray(Wp_f[half * 384 : (half + 1) * 384, :]),
            }
        )
    return in_maps


def kernel(x, Wq, Wk, Wv, Wp, bp, bass_run_kwargs=None, **_unused):
    perm = _unused.pop("perm")
    from concourse.bass_utils import run_bass_kernel_spmd

    x = np.asarray(x, np.float32)
    nc = _build_program()
    in_maps = _make_in_maps(
        x,
        np.asarray(Wq, np.float32),
        np.asarray(Wk, np.float32),
        np.asarray(Wv, np.float32),
        np.asarray(Wp, np.float32),
        np.asarray(perm),
    )
    res = run_bass_kernel_spmd(
        nc, in_maps, core_ids=list(range(8)), **(bass_run_kwargs or {})
    )
    bp = np.asarray(bp, np.float32)
    y = np.empty((B, P, C), np.float32)
    for b in range(B):
        y[b] = res.results[2 * b]["y"] + res.results[2 * b + 1]["y"] + bp
    if bass_run_kwargs:
        kernel.last_results = res
    return y


# revision 12
# speedup vs baseline: 1.3278x; 1.3278x over previous
"""Trainium2 Bass kernel for GQA (nn_GQA_28561532518475).

8 cores = 4 batches x 2 kv-head halves.  perm is folded into the weights on
the host (Wq cols -> slot order * scale, Wk/Wv rows by argsort(perm), Wp rows
by perm), so the device kernel is a plain GQA.

v2 changes vs baseline:
- Scores matmuls for the two grouped q-heads (g=0 rows 0:64, g=1 rows 64:128)
  are emitted adjacently so they run CONCURRENTLY via PE row tiling
  (tile_position auto-derived from base_partition) -> ~2x on the scores GEMM.
- Both heads of a kv group stream through the same i-loop, with exp of the
  [128,1024] score tiles split between the Scalar engine (hardware exp) and
  the Vector engine (Schraudolph exp2 bit-trick: one tensor_scalar affine with
  int32 output, bitcast back to fp32) to break the ACT 1-elem/lane/cycle
  bottleneck (~220us if all exp on ACT).
- v projection moving operand padded to 256 cols (fp32r below 256 runs at
  1/4 rate on a warm PE).
- V for all 3 kv heads lives in one [128, PT, NKV, HD+1] tile with a ones
  column so the attention matmul also emits the softmax denominator.
"""

import numpy as np

B, P, C = 4, 2048, 768
H, HK, HD, GS = 12, 6, 64, 2
SCALE = HD ** -0.5
NKV = 3          # kv heads per core
NH = 6           # q heads per core
KT = C // 128    # 6 contraction tiles
PT = P // 128    # 16 row tiles
QB = 1024        # q-block width for attention
NQB = P // QB    # 2

# Schraudolph exp2 constants (bf16 variant): exp(s) ~= bitcast_bf16(
# round_i16(s*log2e*2^7 + ((127 - 0.0564) * 2^7))).  c = -0.0564 zeroes the
# mean relative error of the mantissa-linear-interp over a uniform
# fractional part.
_EXP_S1 = float(1.4426950408889634 * (1 << 7))
_EXP_S2 = float((127 << 7) - 0.0564 * (1 << 7))

_cached_nc = None


def _build_program():
    global _cached_nc
    if _cached_nc is not None:
        return _cached_nc

    import concourse.bass as bass
    import concourse.mybir as mybir
    import concourse.tile as tile
    from concourse import bacc

    fp32 = mybir.dt.float32
    fp32r = mybir.dt.float32r
    bf16 = mybir.dt.bfloat16
    i16 = mybir.dt.int16
    EXP = mybir.ActivationFunctionType.Exp
    MULT = mybir.AluOpType.mult
    ADD = mybir.AluOpType.add

    nc = bacc.Bacc("TRN2", target_bir_lowering=False, debug=False)
    xT = nc.dram_tensor("xT", [C, P], fp32r, kind="ExternalInput").ap()
    wq = nc.dram_tensor("wq", [C, 384], fp32r, kind="ExternalInput").ap()
    wkd = nc.dram_tensor("wkd", [C, 384], fp32r, kind="ExternalInput").ap()
    wv = nc.dram_tensor("wv", [C, 256], fp32r, kind="ExternalInput").ap()
    wp = nc.dram_tensor("wp", [384, C], fp32r, kind="ExternalInput").ap()
    y = nc.dram_tensor("y", [P, C], fp32, kind="ExternalOutput").ap()
    rcd = nc.dram_tensor("rcd", [NH * NQB, QB], fp32).ap()
    rcd2 = nc.dram_tensor("rcd2", [NH * NQB, QB], fp32).ap()

    with tile.TileContext(nc) as tc:
        from contextlib import ExitStack

        with ExitStack() as ctx:
            wpool = ctx.enter_context(tc.tile_pool(name="weights", bufs=1))
            qkvp = ctx.enter_context(tc.tile_pool(name="qkv", bufs=1))
            outp = ctx.enter_context(tc.tile_pool(name="outT", bufs=1))
            epool = ctx.enter_context(tc.tile_pool(name="E", bufs=4))
            nrmp = ctx.enter_context(tc.tile_pool(name="norm", bufs=1))
            ysbp = ctx.enter_context(tc.tile_pool(name="ysb", bufs=3))

            # x kept as 4 column-quarter tiles so consumers start after the
            # first quarter's DMA instead of the full 6.3MB transfer
            xts = [wpool.tile([128, KT, 512], fp32r, name=f"xt{q}") for q in range(4)]
            for q in range(4):
                for kc in range(KT):
                    nc.sync.dma_start(
                        xts[q][:, kc, :],
                        xT[kc * 128 : (kc + 1) * 128, q * 512 : (q + 1) * 512],
                    )
            wv_sb = wpool.tile([128, KT, 256], fp32r)
            nc.sync.dma_start(wv_sb[:], wv.rearrange("(t p) n -> p t n", p=128))
            wq_sb = wpool.tile([128, KT, 384], fp32r)
            nc.sync.dma_start(wq_sb[:], wq.rearrange("(t p) n -> p t n", p=128))
            wkd_sb = wpool.tile([128, KT, 384], fp32r)
            nc.sync.dma_start(wkd_sb[:], wkd.rearrange("(t p) n -> p t n", p=128))
            wp_sb = wpool.tile([128, 3, C], fp32r)
            nc.sync.dma_start(wp_sb[:], wp.rearrange("(t p) n -> p t n", p=128))

            qts = [qkvp.tile([128, P], fp32r, name=f"qt{m}", tag=f"qt{m}") for m in range(NKV)]
            kts = [qkvp.tile([128, P], fp32r, name=f"kt{m}", tag=f"kt{m}") for m in range(NKV)]
            # all 3 kv heads' V in one tile: [128, PT, NKV, HD+1], col HD = ones
            vex = qkvp.tile([128, PT, NKV, HD + 1], bf16, name="vex", tag="vex")
            nc.vector.memset(vex[:, :, :, HD], 1.0)
            outTs = [outp.tile([128, P], fp32r, name=f"oT{m}", tag=f"oT{m}") for m in range(NKV)]

            expctr = [0]

            with tc.tile_pool(name="mm_ps", bufs=2, space="PSUM") as sps, tc.tile_pool(
                name="o_ps", bufs=1, space="PSUM"
            ) as ops:

                def emit_qk_group(kv, di, nb):
                    w_sb, dest = ((wq_sb, qts[kv]), (wkd_sb, kts[kv]))[di]
                    ps = sps.tile([128, 512], fp32, name="pj", tag="s")
                    for kc in range(KT):
                        nc.tensor.matmul(
                            ps[:],
                            w_sb[:, kc, kv * 128 : (kv + 1) * 128],
                            xts[nb][:, kc, :],
                            start=(kc == 0),
                            stop=(kc == KT - 1),
                        )
                    # alternate the psum->sbuf copies between DVE and ACT
                    if (di * 4 + nb) % 2 == 0:
                        nc.vector.tensor_copy(dest[:, nb * 512 : (nb + 1) * 512], ps[:])
                    else:
                        nc.scalar.copy(dest[:, nb * 512 : (nb + 1) * 512], ps[:])

                def qk_proj(kv):
                    for di in range(2):
                        for nb in range(4):
                            emit_qk_group(kv, di, nb)

                def qk_fillers(kv):
                    # 8 filler units, one qk psum group each
                    for di in range(2):
                        for nb in range(4):
                            yield lambda di=di, nb=nb: emit_qk_group(kv, di, nb)

                def emit_v_group(i):
                    ps = sps.tile([128, 256], fp32, name="vp", tag="s")
                    for kc in range(KT):
                        nc.tensor.matmul(
                            ps[:],
                            xts[i // 4][:, kc, (i % 4) * 128 : (i % 4 + 1) * 128],
                            wv_sb[:, kc, :],
                            start=(kc == 0),
                            stop=(kc == KT - 1),
                        )
                    psv = ps.rearrange("p (h d) -> p h d", h=4)
                    # one strided copy: [128, 3, 64] -> vex[:, i, :, 0:64]
                    nc.scalar.copy(vex[:, i, :, 0:HD], psv[:, 0:NKV, :])

                def emit_exp(e_i16, s_ps, on_dve):
                    if on_dve:
                        nc.vector.tensor_scalar(
                            e_i16[:], s_ps[:], _EXP_S1, _EXP_S2, MULT, ADD
                        )
                    else:
                        nc.scalar.activation(e_i16.bitcast(bf16), s_ps[:], EXP)

                def emit_av(kv, obs, i, es):
                    for nb in range(QB // 512):
                        for g in range(GS):
                            nc.tensor.matmul(
                                obs[g][:, nb * 512 : (nb + 1) * 512],
                                vex[:, i, kv, :],
                                es[g].bitcast(bf16)[:, nb * 512 : (nb + 1) * 512],
                                start=(i == 0),
                                stop=(i == PT - 1),
                            )

                def attention(kv, fillers=()):
                    fillers = iter(fillers)
                    for jq in range(NQB):
                        q0 = jq * QB
                        obs = [
                            ops.tile([HD + 1, QB], fp32, name=f"ob{g}", tag=f"ob{g}")
                            for g in range(GS)
                        ]
                        prev = None
                        for i in range(PT):
                            sts = [
                                sps.tile([128, QB], fp32, name=f"sc{g}", tag="s")
                                for g in range(GS)
                            ]
                            # row-tiled concurrent score matmuls: g=0 on PE
                            # rows 0:64, g=1 on rows 64:128, adjacent in the
                            # instruction stream
                            for nb in range(QB // 512):
                                for g in range(GS):
                                    gp = slice(g * 64, (g + 1) * 64)
                                    nc.tensor.matmul(
                                        sts[g][:, nb * 512 : (nb + 1) * 512],
                                        kts[kv][gp, i * 128 : (i + 1) * 128],
                                        qts[kv][gp, q0 + nb * 512 : q0 + (nb + 1) * 512],
                                        start=True,
                                        stop=True,
                                    )
                            es = []
                            for g in range(GS):
                                e = epool.tile([128, QB], i16, tag="e")
                                # one exp on ACT, one on DVE each i; swap
                                # parity so each head is 50% Schraudolph
                                emit_exp(e, sts[g], on_dve=((g + i) % 2 == 1))
                                es.append(e)
                            # AV for the previous i is emitted after this i's
                            # scores so the PE FIFO has work while exp(i) runs
                            if prev is not None:
                                emit_av(kv, obs, prev[0], prev[1])
                            prev = (i, es)
                            # interleave an independent projection matmul
                            # group as PE filler to keep the HAM clock warm
                            if i % 2 == 1:
                                f = next(fillers, None)
                                if f is not None:
                                    f()
                        emit_av(kv, obs, prev[0], prev[1])
                        # normalize: free the ob psum quickly via an ACT copy
                        # to SBUF, then softmax sums -> DRAM -> lane-spread
                        # reciprocal -> DRAM -> partition-broadcast read, and
                        # a GpSimd multiply (all-SBUF operands)
                        for g in range(GS):
                            ob = obs[g]
                            h = 2 * kv + g
                            u = h * NQB + jq
                            osb = nrmp.tile([HD + 1, QB], fp32, tag=f"osb{g}")
                            if g == 0:
                                nc.scalar.copy(osb[:], ob[:])
                            else:
                                nc.vector.tensor_copy(osb[:], ob[:])
                            nc.sync.dma_start(rcd[u : u + 1, :], osb[HD : HD + 1, :])
                            rr = nrmp.tile([128, QB // 128], fp32, tag="rr")
                            lanes = bass.AP(
                                tensor=rcd.tensor,
                                offset=u * QB,
                                ap=[[QB // 128, 128], [1, QB // 128]],
                            )
                            nc.sync.dma_start(rr[:], lanes)
                            rr2 = nrmp.tile([128, QB // 128], fp32, tag="rr2")
                            nc.vector.reciprocal(rr2[:], rr[:])
                            lanes2 = bass.AP(
                                tensor=rcd2.tensor,
                                offset=u * QB,
                                ap=[[QB // 128, 128], [1, QB // 128]],
                            )
                            nc.sync.dma_start(lanes2, rr2[:])
                            rb = nrmp.tile([HD, QB], fp32, tag="rb")
                            bcast = bass.AP(
                                tensor=rcd2.tensor, offset=u * QB, ap=[[0, HD], [1, QB]]
                            )
                            nc.gpsimd.dma_start(rb[:], bcast)
                            if g == 0:
                                nc.gpsimd.tensor_mul(
                                    outTs[kv][0:HD, q0 : q0 + QB], osb[0:HD, :], rb[:]
                                )
                            else:
                                sc2 = nrmp.tile([HD, QB], fp32r, tag="sc2")
                                nc.gpsimd.tensor_mul(sc2[:], osb[0:HD, :], rb[:])
                                nc.sync.dma_start(
                                    outTs[kv][HD:128, q0 : q0 + QB], sc2[:]
                                )

                import itertools

                def v_fillers():
                    # 2 v groups per filler slot, staying >=2 ahead of the
                    # consuming AV(kv=0) i-loop
                    for i0 in range(4, PT, 2):
                        yield lambda i0=i0: (emit_v_group(i0), emit_v_group(i0 + 1))

                qk_proj(0)
                for i in range(4):
                    emit_v_group(i)
                attention(0, itertools.chain(v_fillers(), qk_fillers(1)))
                # weave gaps between qk(2) groups so they spread across both
                # jq blocks of attention(1)
                attention(
                    1,
                    itertools.chain.from_iterable((f, None) for f in qk_fillers(2)),
                )
                attention(2)

            # ---------------- output projection ----------------
            with tc.tile_pool(name="y_ps", bufs=4, space="PSUM") as yps:
                for mt in range(PT):
                    for nh in range(2):
                        ps = yps.tile([128, 384], fp32, tag="y")
                        for kf in range(3):
                            nc.tensor.matmul(
                                ps[:],
                                outTs[kf][:, mt * 128 : (mt + 1) * 128],
                                wp_sb[:, kf, nh * 384 : (nh + 1) * 384],
                                start=(kf == 0),
                                stop=(kf == 2),
                            )
                        ysb = ysbp.tile([128, 384], fp32, tag="y")
                        nc.vector.tensor_copy(ysb[:], ps[:])
                        nc.sync.dma_start(
                            y[mt * 128 : (mt + 1) * 128, nh * 384 : (nh + 1) * 384],
                            ysb[:],
                        )

    nc.compile()
    _cached_nc = nc
    return nc


def _make_in_maps(x, Wq, Wk, Wv, Wp, perm):
    inv = np.argsort(perm)
    Wq_f = np.ascontiguousarray(
        Wq.reshape(C, H, HD)[:, perm, :].reshape(C, C) * SCALE
    )
    Wk_f = np.ascontiguousarray(Wk.reshape(H, HD, HK * HD)[inv].reshape(C, HK * HD))
    Wv_f = np.ascontiguousarray(Wv.reshape(H, HD, HK * HD)[inv].reshape(C, HK * HD))
    Wp_f = np.ascontiguousarray(Wp.reshape(H, HD, C)[perm].reshape(C, C))

    in_maps = []
    for core in range(8):
        b, half = core // 2, core % 2
        wk_half = Wk_f[:, half * 192 : (half + 1) * 192].reshape(C, NKV, 1, HD)
        wkd = np.ascontiguousarray(
            np.broadcast_to(wk_half, (C, NKV, 2, HD)).reshape(C, 384)
        )
        wv_pad = np.zeros((C, 256), np.float32)
        wv_pad[:, :192] = Wv_f[:, half * 192 : (half + 1) * 192]
        in_maps.append(
            {
                "xT": np.ascontiguousarray(x[b].T),
                "wq": np.ascontiguousarray(Wq_f[:, half * 384 : (half + 1) * 384]),
                "wkd": wkd,
                "wv": wv_pad,
                "wp": np.ascontiguousarray(Wp_f[half * 384 : (half + 1) * 384, :]),
            }
        )
    return in_maps


def kernel(x, Wq, Wk, Wv, Wp, bp, bass_run_kwargs=None, **_unused):
    perm = _unused.pop("perm")
    from concourse.bass_utils import run_bass_kernel_spmd

    x = np.asarray(x, np.float32)
    nc = _build_program()
    in_maps = _make_in_maps(
        x,
        np.asarray(Wq, np.float32),
        np.asarray(Wk, np.float32),
        np.asarray(Wv, np.float32),
        np.asarray(Wp, np.float32),
        np.asarray(perm),
    )
    res = run_bass_kernel_spmd(
        nc, in_maps, core_ids=list(range(8)), **(bass_run_kwargs or {})
    )
    bp = np.asarray(bp, np.float32)
    y = np.empty((B, P, C), np.float32)
    for b in range(B):
        y[b] = res.results[2 * b]["y"] + res.results[2 * b + 1]["y"] + bp
    if bass_run_kwargs:
        kernel.last_results = res
    return y


# revision 14
# speedup vs baseline: 1.5844x; 1.1932x over previous
"""Trainium2 Bass kernel for GQA (nn_GQA_28561532518475).

8 cores = 4 batches x 2 kv-head halves.  perm is folded into the weights on
the host (Wq cols -> slot order * scale, Wk/Wv rows by argsort(perm), Wp rows
by perm), so the device kernel is a plain GQA.

v2 changes vs baseline:
- Scores matmuls for the two grouped q-heads (g=0 rows 0:64, g=1 rows 64:128)
  are emitted adjacently so they run CONCURRENTLY via PE row tiling
  (tile_position auto-derived from base_partition) -> ~2x on the scores GEMM.
- Both heads of a kv group stream through the same i-loop, with exp of the
  [128,1024] score tiles split between the Scalar engine (hardware exp) and
  the Vector engine (Schraudolph exp2 bit-trick: one tensor_scalar affine with
  int32 output, bitcast back to fp32) to break the ACT 1-elem/lane/cycle
  bottleneck (~220us if all exp on ACT).
- v projection moving operand padded to 256 cols (fp32r below 256 runs at
  1/4 rate on a warm PE).
- V for all 3 kv heads lives in one [128, PT, NKV, HD+1] tile with a ones
  column so the attention matmul also emits the softmax denominator.
"""

import numpy as np

B, P, C = 4, 2048, 768
H, HK, HD, GS = 12, 6, 64, 2
SCALE = HD ** -0.5
NKV = 3          # kv heads per core
NH = 6           # q heads per core
KT = C // 128    # 6 contraction tiles
PT = P // 128    # 16 row tiles
QB = 1024        # q-block width for attention
NQB = P // QB    # 2

# Schraudolph exp2 constants (bf16 variant): exp(s) ~= bitcast_bf16(
# round_i16(s*log2e*2^7 + ((127 - 0.0564) * 2^7))).  c = -0.0564 zeroes the
# mean relative error of the mantissa-linear-interp over a uniform
# fractional part.
_EXP_S1 = float(1.4426950408889634 * (1 << 7))
_EXP_S2 = float((127 << 7) - 0.0564 * (1 << 7))

_cached_nc = None


def _build_program():
    global _cached_nc
    if _cached_nc is not None:
        return _cached_nc

    import concourse.bass as bass
    import concourse.mybir as mybir
    import concourse.tile as tile
    from concourse import bacc

    fp32 = mybir.dt.float32
    fp32r = mybir.dt.float32r
    bf16 = mybir.dt.bfloat16
    i16 = mybir.dt.int16
    EXP = mybir.ActivationFunctionType.Exp
    MULT = mybir.AluOpType.mult
    ADD = mybir.AluOpType.add

    nc = bacc.Bacc("TRN2", target_bir_lowering=False, debug=False)
    xT = nc.dram_tensor("xT", [C, P], bf16, kind="ExternalInput").ap()
    wq = nc.dram_tensor("wq", [C, 384], bf16, kind="ExternalInput").ap()
    wkd = nc.dram_tensor("wkd", [C, 384], bf16, kind="ExternalInput").ap()
    wv = nc.dram_tensor("wv", [C, 256], bf16, kind="ExternalInput").ap()
    wp = nc.dram_tensor("wp", [384, C], fp32r, kind="ExternalInput").ap()
    y = nc.dram_tensor("y", [P, C], fp32, kind="ExternalOutput").ap()
    rcd = nc.dram_tensor("rcd", [NH * NQB, QB], fp32).ap()
    rcd2 = nc.dram_tensor("rcd2", [NH * NQB, QB], fp32).ap()

    with tile.TileContext(nc) as tc:
        from contextlib import ExitStack

        with ExitStack() as ctx:
            wpool = ctx.enter_context(tc.tile_pool(name="weights", bufs=1))
            qkvp = ctx.enter_context(tc.tile_pool(name="qkv", bufs=1))
            outp = ctx.enter_context(tc.tile_pool(name="outT", bufs=1))
            epool = ctx.enter_context(tc.tile_pool(name="E", bufs=6))
            nrmp = ctx.enter_context(tc.tile_pool(name="norm", bufs=2))
            ysbp = ctx.enter_context(tc.tile_pool(name="ysb", bufs=3))

            # x kept as 4 column-quarter tiles so consumers start after the
            # first quarter's DMA instead of the full 6.3MB transfer
            xts = [wpool.tile([128, KT, 512], bf16, name=f"xt{q}") for q in range(4)]
            for q in range(4):
                for kc in range(KT):
                    nc.sync.dma_start(
                        xts[q][:, kc, :],
                        xT[kc * 128 : (kc + 1) * 128, q * 512 : (q + 1) * 512],
                    )
            wv_sb = wpool.tile([128, KT, 256], bf16)
            nc.sync.dma_start(wv_sb[:], wv.rearrange("(t p) n -> p t n", p=128))
            wq_sb = wpool.tile([128, KT, 384], bf16)
            nc.sync.dma_start(wq_sb[:], wq.rearrange("(t p) n -> p t n", p=128))
            wkd_sb = wpool.tile([128, KT, 384], bf16)
            nc.sync.dma_start(wkd_sb[:], wkd.rearrange("(t p) n -> p t n", p=128))
            wp_sb = wpool.tile([128, 3, C], fp32r)
            nc.sync.dma_start(wp_sb[:], wp.rearrange("(t p) n -> p t n", p=128))

            qts = [qkvp.tile([128, P], bf16, name=f"qt{m}", tag=f"qt{m}") for m in range(NKV)]
            kts = [qkvp.tile([128, P], bf16, name=f"kt{m}", tag=f"kt{m}") for m in range(NKV)]
            # all 3 kv heads' V in one tile: [128, PT, NKV, HD+1], col HD = ones
            vex = qkvp.tile([128, PT, NKV, HD + 1], bf16, name="vex", tag="vex")
            nc.vector.memset(vex[:, :, :, HD], 1.0)
            outTs = [outp.tile([128, P], fp32r, name=f"oT{m}", tag=f"oT{m}") for m in range(NKV)]

            expctr = [0]

            with tc.tile_pool(name="mm_ps", bufs=2, space="PSUM") as sps, tc.tile_pool(
                name="o_ps", bufs=1, space="PSUM"
            ) as ops:

                def emit_qk_group(kv, di, nb):
                    w_sb, dest = ((wq_sb, qts[kv]), (wkd_sb, kts[kv]))[di]
                    ps = sps.tile([128, 512], fp32, name="pj", tag="s")
                    for kc in range(KT):
                        nc.tensor.matmul(
                            ps[:],
                            w_sb[:, kc, kv * 128 : (kv + 1) * 128],
                            xts[nb][:, kc, :],
                            start=(kc == 0),
                            stop=(kc == KT - 1),
                        )
                    # alternate the psum->sbuf copies between DVE and ACT
                    if (di * 4 + nb) % 2 == 0:
                        nc.vector.tensor_copy(dest[:, nb * 512 : (nb + 1) * 512], ps[:])
                    else:
                        nc.scalar.copy(dest[:, nb * 512 : (nb + 1) * 512], ps[:])

                def qk_proj(kv):
                    for di in range(2):
                        for nb in range(4):
                            emit_qk_group(kv, di, nb)

                def qk_fillers(kv):
                    # 8 filler units, one qk psum group each
                    for di in range(2):
                        for nb in range(4):
                            yield lambda di=di, nb=nb: emit_qk_group(kv, di, nb)

                def emit_v_group(i):
                    ps = sps.tile([128, 256], fp32, name="vp", tag="s")
                    for kc in range(KT):
                        nc.tensor.matmul(
                            ps[:],
                            xts[i // 4][:, kc, (i % 4) * 128 : (i % 4 + 1) * 128],
                            wv_sb[:, kc, :],
                            start=(kc == 0),
                            stop=(kc == KT - 1),
                        )
                    psv = ps.rearrange("p (h d) -> p h d", h=4)
                    # one strided copy: [128, 3, 64] -> vex[:, i, :, 0:64]
                    nc.scalar.copy(vex[:, i, :, 0:HD], psv[:, 0:NKV, :])

                def emit_out_chunk(mt, nh):
                    ps = sps.tile([128, 384], fp32, name="yp", tag="s")
                    for kf in range(3):
                        nc.tensor.matmul(
                            ps[:],
                            outTs[kf][:, mt * 128 : (mt + 1) * 128],
                            wp_sb[:, kf, nh * 384 : (nh + 1) * 384],
                            start=(kf == 0),
                            stop=(kf == 2),
                        )
                    ysb = ysbp.tile([128, 384], fp32, tag="y")
                    if (mt + nh) % 2 == 0:
                        nc.vector.tensor_copy(ysb[:], ps[:])
                    else:
                        nc.scalar.copy(ysb[:], ps[:])
                    nc.sync.dma_start(
                        y[mt * 128 : (mt + 1) * 128, nh * 384 : (nh + 1) * 384],
                        ysb[:],
                    )

                def emit_exp(e_i16, s_ps, on_dve):
                    if on_dve:
                        nc.vector.tensor_scalar(
                            e_i16[:], s_ps[:], _EXP_S1, _EXP_S2, MULT, ADD
                        )
                    else:
                        nc.scalar.activation(e_i16.bitcast(bf16), s_ps[:], EXP)

                def emit_av(kv, obs, i, es):
                    for nb in range(QB // 512):
                        for g in range(GS):
                            nc.tensor.matmul(
                                obs[g][:, nb * 512 : (nb + 1) * 512],
                                vex[:, i, kv, :],
                                es[g].bitcast(bf16)[:, nb * 512 : (nb + 1) * 512],
                                start=(i == 0),
                                stop=(i == PT - 1),
                            )

                def attention(kv, fillers=()):
                    fillers = iter(fillers)
                    for jq in range(NQB):
                        q0 = jq * QB
                        obs = [
                            ops.tile([HD + 1, QB], fp32, name=f"ob{g}", tag=f"ob{g}")
                            for g in range(GS)
                        ]
                        prev = None
                        for i in range(PT):
                            sts = [
                                sps.tile([128, QB], fp32, name=f"sc{g}", tag="s")
                                for g in range(GS)
                            ]
                            # row-tiled concurrent score matmuls: g=0 on PE
                            # rows 0:64, g=1 on rows 64:128, adjacent in the
                            # instruction stream
                            for g in range(GS):
                                gp = slice(g * 64, (g + 1) * 64)
                                for nb in range(QB // 512):
                                    nc.tensor.matmul(
                                        sts[g][:, nb * 512 : (nb + 1) * 512],
                                        kts[kv][gp, i * 128 : (i + 1) * 128],
                                        qts[kv][gp, q0 + nb * 512 : q0 + (nb + 1) * 512],
                                        start=True,
                                        stop=True,
                                    )
                            es = []
                            for g in range(GS):
                                e = epool.tile([128, QB], i16, tag="e")
                                # one exp on ACT, one on DVE each i; swap
                                # parity so each head is 50% Schraudolph
                                emit_exp(e, sts[g], on_dve=((g + i) % 2 == 1))
                                es.append(e)
                            # AV for the previous i is emitted after this i's
                            # scores so the PE FIFO has work while exp(i) runs
                            if prev is not None:
                                emit_av(kv, obs, prev[0], prev[1])
                            prev = (i, es)
                            # interleave an independent projection matmul
                            # group as PE filler to keep the HAM clock warm
                            f = next(fillers, None)
                            if f is not None:
                                f()
                        emit_av(kv, obs, prev[0], prev[1])
                        # normalize: free the ob psum quickly via an ACT copy
                        # to SBUF, then softmax sums -> DRAM -> lane-spread
                        # reciprocal -> DRAM -> partition-broadcast read, and
                        # a GpSimd multiply (all-SBUF operands)
                        for g in range(GS):
                            ob = obs[g]
                            h = 2 * kv + g
                            u = h * NQB + jq
                            osb = nrmp.tile([HD + 1, QB], fp32, tag=f"osb{g}")
                            if g == 0:
                                nc.scalar.copy(osb[:], ob[:])
                            else:
                                nc.vector.tensor_copy(osb[:], ob[:])
                            nc.sync.dma_start(rcd[u : u + 1, :], osb[HD : HD + 1, :])
                            rr = nrmp.tile([128, QB // 128], fp32, tag="rr")
                            lanes = bass.AP(
                                tensor=rcd.tensor,
                                offset=u * QB,
                                ap=[[QB // 128, 128], [1, QB // 128]],
                            )
                            nc.sync.dma_start(rr[:], lanes)
                            rr2 = nrmp.tile([128, QB // 128], fp32, tag="rr2")
                            nc.vector.reciprocal(rr2[:], rr[:])
                            lanes2 = bass.AP(
                                tensor=rcd2.tensor,
                                offset=u * QB,
                                ap=[[QB // 128, 128], [1, QB // 128]],
                            )
                            nc.sync.dma_start(lanes2, rr2[:])
                            rb = nrmp.tile([HD, QB], fp32, tag="rb")
                            bcast = bass.AP(
                                tensor=rcd2.tensor, offset=u * QB, ap=[[0, HD], [1, QB]]
                            )
                            nc.gpsimd.dma_start(rb[:], bcast)
                            if g == 0:
                                nc.gpsimd.tensor_mul(
                                    outTs[kv][0:HD, q0 : q0 + QB], osb[0:HD, :], rb[:]
                                )
                            else:
                                sc2 = nrmp.tile([HD, QB], fp32r, tag="sc2")
                                nc.gpsimd.tensor_mul(sc2[:], osb[0:HD, :], rb[:])
                                nc.sync.dma_start(
                                    outTs[kv][HD:128, q0 : q0 + QB], sc2[:]
                                )

                import itertools

                def v_fillers():
                    # 2 v groups per filler slot, staying >=2 ahead of the
                    # consuming AV(kv=0) i-loop
                    for i0 in range(4, PT, 2):
                        yield lambda i0=i0: (emit_v_group(i0), emit_v_group(i0 + 1))

                qk_proj(0)
                for i in range(4):
                    emit_v_group(i)
                attention(0, itertools.chain(v_fillers(), qk_fillers(1)))
                # weave gaps between qk(2) groups so they spread across both
                # jq blocks of attention(1)
                attention(
                    1,
                    itertools.chain.from_iterable((f, None) for f in qk_fillers(2)),
                )
                # out_proj chunks over the first q-half become PE filler for
                # attention(2)'s second q-half (their outTs inputs complete
                # once (kv=2, jq=0) is normalized)
                of = iter([(mt, nh) for mt in range(8) for nh in range(2)])

                def out_filler_gen():
                    for mt, nh in of:
                        yield lambda mt=mt, nh=nh: emit_out_chunk(mt, nh)

                attention(
                    2,
                    itertools.chain(itertools.repeat(None, 20), out_filler_gen()),
                )
                for mt, nh in of:
                    emit_out_chunk(mt, nh)
                for mt in range(8, PT):
                    for nh in range(2):
                        emit_out_chunk(mt, nh)


    nc.compile()
    _cached_nc = nc
    return nc


def _make_in_maps(x, Wq, Wk, Wv, Wp, perm):
    from ml_dtypes import bfloat16

    inv = np.argsort(perm)
    Wq_f = np.ascontiguousarray(
        Wq.reshape(C, H, HD)[:, perm, :].reshape(C, C) * SCALE
    )
    Wk_f = np.ascontiguousarray(Wk.reshape(H, HD, HK * HD)[inv].reshape(C, HK * HD))
    Wv_f = np.ascontiguousarray(Wv.reshape(H, HD, HK * HD)[inv].reshape(C, HK * HD))
    Wp_f = np.ascontiguousarray(Wp.reshape(H, HD, C)[perm].reshape(C, C))

    in_maps = []
    for core in range(8):
        b, half = core // 2, core % 2
        wk_half = Wk_f[:, half * 192 : (half + 1) * 192].reshape(C, NKV, 1, HD)
        wkd = np.ascontiguousarray(
            np.broadcast_to(wk_half, (C, NKV, 2, HD)).reshape(C, 384)
        )
        wv_pad = np.zeros((C, 256), bfloat16)
        wv_pad[:, :192] = Wv_f[:, half * 192 : (half + 1) * 192]
        in_maps.append(
            {
                "xT": np.ascontiguousarray(x[b].T.astype(bfloat16)),
                "wq": np.ascontiguousarray(
                    Wq_f[:, half * 384 : (half + 1) * 384].astype(bfloat16)
                ),
                "wkd": wkd.astype(bfloat16),
                "wv": wv_pad,
                "wp": np.ascontiguousarray(Wp_f[half * 384 : (half + 1) * 384, :]),
            }
        )
    return in_maps


def kernel(x, Wq, Wk, Wv, Wp, bp, bass_run_kwargs=None, **_unused):
    perm = _unused.pop("perm")
    from concourse.bass_utils import run_bass_kernel_spmd

    x = np.asarray(x, np.float32)
    nc = _build_program()
    in_maps = _make_in_maps(
        x,
        np.asarray(Wq, np.float32),
        np.asarray(Wk, np.float32),
        np.asarray(Wv, np.float32),
        np.asarray(Wp, np.float32),
        np.asarray(perm),
    )
    res = run_bass_kernel_spmd(
        nc, in_maps, core_ids=list(range(8)), **(bass_run_kwargs or {})
    )
    bp = np.asarray(bp, np.float32)
    y = np.empty((B, P, C), np.float32)
    for b in range(B):
        y[b] = res.results[2 * b]["y"] + res.results[2 * b + 1]["y"] + bp
    if bass_run_kwargs:
        kernel.last_results = res
    return y
